# revision 1
# baseline (speedup 1.0000x reference)
"""NSVQ (noise-substitution VQ) Trainium2 kernel.

Problem: out = decode(x + ||x - c_nearest|| / (||r||+eps) * r), where
x = encode(input). Key identity used: ||x - c_nearest||^2 =
||x||^2 - 2*max_k(x.c_k - 0.5||c_k||^2), so no argmin / gather is needed.

Sharding: data-parallel over tokens. Core i handles batches [2i, 2i+1]
(4096 tokens each); codebook + projection weights replicated.

Layout per core (tokens chunked 128 at a time, tiled 512 at a time):
  encode:  x_T[64, 512] = W_in^T @ inp (PSUM, float32r matmuls)
  xhat:    SBUF copy of x_T + b_in, with a ones row (row 64) so the
           distance matmul folds -0.5||c||^2 via an augmented codebook.
  dist:    scores [128 tok, 1024 codes] in one 2-bank PSUM tile (two
           float32r matmuls vs the augmented codebook), then a single
           DVE reduce_max -> smax per chunk. (tensor_tensor_reduce and
           Pool-engine free-axis reduce are broken on this HW/toolchain.)
  norms:   ||x||^2: 4 PE transposes/tile -> one ACT Square -> one DVE
           3D reduce_sum; ||r||^2: DVE STT r*r with sum-accumulate.
  scale:   sqrt(relu(nsq - 2 smax) * recip(rsq)), batched per tile.
  decode:  scaled r (GPSIMD mult, stride-0 broadcast of scale) is
           transposed on PE accumulating onto x_T PSUM -> q_T;
           out = W_out_aug^T @ [q_T; ones] (b_out folded).
"""

import numpy as np
from contextlib import ExitStack

B, DIM, T = 16, 256, 2048
K, D = 1024, 64
NCORES = 8
BPC = B // NCORES          # batches per core
NTOK = BPC * T             # tokens per core
TTILE = 512                # tokens per tile
NTILES = NTOK // TTILE     # 8
CHUNK = 128
CPT = TTILE // CHUNK       # chunks per tile = 4
NCHUNK = NTOK // CHUNK     # 32
BATCH_TILES = 1            # tiles per scale batch
EPS = 1e-12

_CACHE = {}

import os
ABLATE = set(os.environ.get("KABLATE", "").split(",")) - {""}


def _emit(ctx, tc, aps):
    import concourse.bass as bass
    from concourse import mybir

    nc = tc.nc
    f32 = mybir.dt.float32
    f32r = mybir.dt.float32r
    AX = mybir.AluOpType
    AF = mybir.ActivationFunctionType
    ts = bass.ts

    inp, rr, win, binc, cba, woa, eye, out = (
        aps["inp"], aps["rr"], aps["win"], aps["binc"], aps["cba"],
        aps["woa"], aps["eye"], aps["out"],
    )

    # ---- pools ----
    const = ctx.enter_context(tc.tile_pool(name="const", bufs=1))
    persist = ctx.enter_context(tc.tile_pool(name="persist", bufs=1))
    inpool = ctx.enter_context(tc.tile_pool(name="inpool", bufs=4))
    d1pool = ctx.enter_context(tc.tile_pool(name="d1pool", bufs=5))
    scrpool = ctx.enter_context(tc.tile_pool(name="scrpool", bufs=2))
    sqpool = ctx.enter_context(tc.tile_pool(name="sqpool", bufs=4))
    srpool = ctx.enter_context(tc.tile_pool(name="srpool", bufs=3))
    opool = ctx.enter_context(tc.tile_pool(name="opool", bufs=3))

    xpsum = ctx.enter_context(tc.tile_pool(name="xpsum", bufs=2, space="PSUM"))
    dpsum = ctx.enter_context(tc.tile_pool(name="dpsum", bufs=2, space="PSUM"))
    tpsum = ctx.enter_context(tc.tile_pool(name="tpsum", bufs=1, space="PSUM"))
    opsum = ctx.enter_context(tc.tile_pool(name="opsum", bufs=1, space="PSUM"))

    # ---- constants ----
    w0 = const.tile([128, D], f32r, tag="w0")
    nc.sync.dma_start(w0[:], win[0:128, :])
    w1 = const.tile([128, D], f32r, tag="w1")
    nc.sync.dma_start(w1[:], win[128:256, :])
    binc_sb = const.tile([D, 1], f32, tag="binc")
    nc.sync.dma_start(binc_sb[:], binc[:])
    cba_sb = const.tile([D + 1, K], f32r, tag="cba")
    nc.sync.dma_start(cba_sb[:], cba[:])
    woa_sb = const.tile([D + 1, DIM], f32r, tag="woa")
    nc.sync.dma_start(woa_sb[:], woa[:])
    eye_sb = const.tile([128, 128], f32r, tag="eye")
    nc.sync.dma_start(eye_sb[:], eye[:])

    # whole random-vector slice, token-major [128, NCHUNK, 64]
    rall = persist.tile([128, NCHUNK, D], f32, tag="rall")
    nc.sync.dma_start(rall[:], rr[:])

    # persistent x-hat / q-hat tiles with a ones row at row 64
    xh = [persist.tile([D + 1, TTILE], f32r, tag=f"xh{n}", name=f"xh{n}")
          for n in range(4)]
    qh = [persist.tile([D + 1, TTILE], f32r, tag=f"qh{n}", name=f"qh{n}")
          for n in range(4)]
    for t_ in xh + qh:
        nc.gpsimd.memset(t_[D:D + 1, :].bitcast(f32), 1.0)

    zeros32 = persist.tile([128, NCHUNK], f32, tag="zeros32", name="zeros32")
    nc.gpsimd.memset(zeros32[:], 0.0)

    # per-token stats, chunk j lives in column j
    stats = {}
    for nm in ("smax", "nsq", "rsq", "resid2", "nres", "nrand", "recd",
               "scalev"):
        stats[nm] = persist.tile([128, NCHUNK], f32, tag=nm, name=nm)
    smax, nsq, rsq = stats["smax"], stats["nsq"], stats["rsq"]
    resid2, nres, nrand = stats["resid2"], stats["nres"], stats["nrand"]
    recd, scalev = stats["recd"], stats["scalev"]

    Xtiles = {}

    def phase_a(i):
        b, t4 = divmod(i, NTILES // BPC)
        t0 = t4 * TTILE
        xt = xh[i % 4]

        in0 = inpool.tile([128, TTILE], f32r, tag="in0")
        nc.sync.dma_start(in0[:], inp[b, 0:128, t0:t0 + TTILE])
        in1 = inpool.tile([128, TTILE], f32r, tag="in1")
        nc.sync.dma_start(in1[:], inp[b, 128:256, t0:t0 + TTILE])

        X = xpsum.tile([D, TTILE], f32, tag="X")
        Xtiles[i] = X
        nc.tensor.matmul(X[:], w0[:], in0[:],
                         start=True, stop=False)
        nc.tensor.matmul(X[:], w1[:], in1[:],
                         start=False, stop=True)
        # evacuate + add b_in; row 64 of xt stays = ones
        nc.scalar.activation(xt[0:D, :], X[:], AF.Identity, bias=binc_sb[:])

        for j4 in range(CPT):
            j = CPT * i + j4
            xsl = xt[0:D + 1, ts(j4, CHUNK)]
            # pair-folded scores: E = A + |Dm| = max(s_2k, s_2k+1), then
            # one DVE max-reduce over 512 pairs
            dd = dpsum.tile([128, K], f32, tag="d")
            nc.tensor.matmul(dd[:, 0:K // 2], xsl, cba_sb[:, 0:K // 2],
                             start=True, stop=True)
            nc.tensor.matmul(dd[:, K // 2:K], xsl, cba_sb[:, K // 2:K],
                             start=True, stop=True)
            nc.vector.reduce_max(smax[:, j:j + 1], dd[:],
                                 axis=mybir.AxisListType.X)



        # ||r||^2 batched per tile: GP squares -> one DVE 3D reduce-sum
        if "rsq" not in ABLATE:
            rsqsq = sqpool.tile([128, CPT * D], f32, tag="rsqsq")
            rsl = rall[:, ts(i, CPT), :]
            nc.gpsimd.tensor_tensor(
                rsqsq[:].rearrange("p (c d) -> p c d", c=CPT), rsl, rsl,
                op=AX.mult)
            nc.vector.reduce_sum(
                rsq[:, ts(i, CPT)],
                rsqsq[:].rearrange("p (c d) -> p c d", c=CPT),
                axis=mybir.AxisListType.X)

        # ||x||^2: 4 PE transposes -> per-chunk ACT Square with accumulate
        if "nsq" not in ABLATE:
            XT4 = tpsum.tile([128, CPT * D], f32, tag="xtm")
            for j4 in range(CPT):
                nc.tensor.transpose(XT4[:, ts(j4, D)].bitcast(f32r),
                                    xt[0:D, ts(j4, CHUNK)], eye_sb[0:D, 0:D])
            sq4 = sqpool.tile([128, CPT * D], f32, tag="sq4")
            for j4 in range(CPT):
                nc.scalar.activation(sq4[:, ts(j4, D)], XT4[:, ts(j4, D)],
                                     AF.Square,
                                     accum_out=nsq[:, CPT * i + j4:CPT * i + j4 + 1])

    def scale_math(k):
        if "smath" in ABLATE:
            return
        c8 = ts(k, BATCH_TILES * CPT)
        # recip(rsq) does not depend on smax -> off the critical path
        nc.vector.reciprocal(recd[:, c8], rsq[:, c8])
        # resid^2 = nsq - 2*smax, clamped at 0
        nc.vector.scalar_tensor_tensor(resid2[:, c8], smax[:, c8], -2.0,
                                       nsq[:, c8], AX.mult, AX.add)
        nc.vector.tensor_scalar_max(resid2[:, c8], resid2[:, c8], 0.0)
        nc.gpsimd.tensor_tensor(nres[:, c8], resid2[:, c8], recd[:, c8],
                                op=AX.mult)
        nc.scalar.sqrt(scalev[:, c8], nres[:, c8])

    def phase_b(i):
        b, t4 = divmod(i, NTILES // BPC)
        t0 = t4 * TTILE
        X = Xtiles.pop(i)
        qt = qh[i % 4]

        # scaled r, one TT op per tile: broadcast scale along d via stride-0
        srt = srpool.tile([128, CPT, D], f32r, tag="srt")
        if "srt" not in ABLATE:
            scl = scalev[:, ts(i, CPT)].unsqueeze(2).broadcast_to([128, CPT, D])
            nc.gpsimd.tensor_tensor(srt[:], rall[:, ts(i, CPT), :], scl,
                                    op=AX.mult)
        # transpose-accumulate onto X: q_T = x_T + (scale*r)^T
        if "taccum" not in ABLATE:
            for j4 in range(CPT):
                nc.tensor.matmul(X[:, ts(j4, CHUNK)].bitcast(f32r),
                                 srt[:, j4, :], eye_sb[:], is_transpose=True,
                                 start=False, stop=(j4 == CPT - 1),
                                 skip_group_check=True)
        nc.scalar.activation(qt[0:D, :], X[:], AF.Identity, bias=binc_sb[:])

        osb = opool.tile([128, 2 * TTILE], f32, tag="osb")
        for m in range(2):
            O = opsum.tile([128, TTILE], f32, tag="O", name="O")
            nc.tensor.matmul(O[:], woa_sb[:, ts(m, 128)], qt[:],
                             start=True, stop=True)
            nc.scalar.copy(osb[:, ts(m, TTILE)], O[:])
        nc.sync.dma_start(out[b, 0:128, t0:t0 + TTILE], osb[:, 0:TTILE])
        nc.sync.dma_start(out[b, 128:256, t0:t0 + TTILE], osb[:, TTILE:2 * TTILE])

    for _rep in range(int(os.environ.get("KREPEAT", "1"))):
        for k in range(NTILES // BATCH_TILES):
            for i in range(k * BATCH_TILES, (k + 1) * BATCH_TILES):
                phase_a(i)
            scale_math(k)
            for i in range(k * BATCH_TILES, (k + 1) * BATCH_TILES):
                phase_b(i)


def build():
    if "nc" in _CACHE:
        return _CACHE["nc"]
    from concourse import bacc, mybir
    import concourse.tile as tile

    nc = bacc.Bacc("TRN2", target_bir_lowering=False, debug=False,
                   enable_asserts=False, num_devices=NCORES)
    f32 = mybir.dt.float32
    f32r = mybir.dt.float32r
    aps = {
        "inp": nc.dram_tensor("inp", [BPC, DIM, T], f32r,
                              kind="ExternalInput").ap(),
        "rr": nc.dram_tensor("rr", [128, NCHUNK, D], f32,
                             kind="ExternalInput").ap(),
        "win": nc.dram_tensor("win", [DIM, D], f32r, kind="ExternalInput").ap(),
        "binc": nc.dram_tensor("binc", [D, 1], f32, kind="ExternalInput").ap(),
        "cba": nc.dram_tensor("cba", [D + 1, K], f32r,
                              kind="ExternalInput").ap(),
        "woa": nc.dram_tensor("woa", [D + 1, DIM], f32r,
                              kind="ExternalInput").ap(),
        "eye": nc.dram_tensor("eye", [128, 128], f32r,
                              kind="ExternalInput").ap(),
        "out": nc.dram_tensor("out", [BPC, DIM, T], f32,
                              kind="ExternalOutput").ap(),
    }
    with tile.TileContext(nc) as tc:
        with ExitStack() as ctx:
            _emit(ctx, tc, aps)
    nc.compile()
    _CACHE["nc"] = nc
    return nc


def make_in_maps(input_data, codebooks, W_in, b_in, W_out, b_out,
                 random_vector):
    f = np.float32
    cb = np.asarray(codebooks, f)
    cba = np.concatenate([cb.T, (-0.5 * (cb * cb).sum(1))[None, :]],
                         0).astype(f)  # [65, K] augmented codebook
    woa = np.concatenate([np.asarray(W_out, f),
                          np.asarray(b_out, f)[None, :]], 0).astype(f)
    eye = np.eye(128, dtype=f)
    binc = np.ascontiguousarray(np.asarray(b_in, f).reshape(D, 1))
    win = np.ascontiguousarray(np.asarray(W_in, f))
    rv = np.asarray(random_vector, f).reshape(NCORES, NCHUNK, 128, D)
    in_maps = []
    for i in range(NCORES):
        rr = np.ascontiguousarray(rv[i].transpose(1, 0, 2))  # [128, NCHUNK, D]
        in_maps.append({
            "inp": np.ascontiguousarray(input_data[BPC * i:BPC * (i + 1)],
                                        dtype=f),
            "rr": rr,
            "win": win, "binc": binc, "cba": cba, "woa": woa, "eye": eye,
        })
    return in_maps


def kernel(input_data, codebooks, W_in, b_in, W_out, b_out, random_vector,
           **kwargs):
    from concourse.bass_utils import run_bass_kernel_spmd

    nc = build()
    in_maps = make_in_maps(input_data, codebooks, W_in, b_in, W_out, b_out,
                           random_vector)
    res = run_bass_kernel_spmd(nc, in_maps, core_ids=list(range(NCORES)),
                               **kwargs)
    out = np.concatenate([res.results[i]["out"] for i in range(NCORES)],
                         axis=0)
    _CACHE["last_res"] = res
    return out


if __name__ == "__main__":
    nc = build()
    print("compiled OK")



# revision 3
# speedup vs baseline: 1.0093x; 1.0093x over previous
"""NSVQ (noise-substitution VQ) Trainium2 kernel, v4.

out = decode(x + ||x - c_nearest|| * rhat), rhat = r/(||r||+eps) host-
precomputed, x = encode(input). ||x - c_n||^2 = ||x||^2 - 2 smax where
smax = max_k (x.c_k - 0.5||c_k||^2) -- no argmin / gather needed.

Max strategy (DVE is the only engine that can max; gpsimd/Pool ALU only
implements Add/Multiply; TensorTensor with 2 PSUM operands is rejected
by walrus): use the pair identity
    max(s_even, s_odd) = s_even + relu(s_odd - s_even)
where s_odd - s_even is linear in [x; 1], so the dist matmul emits
[s_even | s_diff] via a pre-paired codebook. ACT computes relu(s_diff)
(PSUM -> SBUF bf16), PE accumulates it back onto the s_even PSUM half
(eye matmul, start=False), DVE does one 512-wide reduce_max. Chunks can
alternatively take the plain path (full 1024-wide DVE reduce_max of an
unpaired codebook) -- the KACT_CHUNKS set balances ACT vs DVE load.

Everything bf16 on-chip (inputs cast on host); output DMA'd bf16 and
cast back to f32 on host. Data-parallel over tokens: core i handles
batches [2i, 2i+1]; codebook + projection weights replicated.
"""

import numpy as np
from contextlib import ExitStack

B, DIM, T = 16, 256, 2048
K, D = 1024, 64
NCORES = 8
BPC = B // NCORES          # batches per core
NTOK = BPC * T             # tokens per core
TTILE = 512                # tokens per tile
NTILES = NTOK // TTILE     # 8
CHUNK = 128
CPT = TTILE // CHUNK       # chunks per tile = 4
NCHUNK = NTOK // CHUNK     # 32
EPS = 1e-12

_CACHE = {}

import os
ABLATE = set(os.environ.get("KABLATE", "").split(",")) - {""}
# chunk indices (within tile) using the ACT-relu pair path; rest use the
# plain full-reduce path on DVE
ACT_CHUNKS = set(int(c) for c in os.environ.get("KACT_CHUNKS", "0,1,2").split(",")
                 if c != "")
# osb evacuation engines for the two output halves: a=ACT, v=DVE
OSB_ENG = os.environ.get("KOSB", "aa")


def _emit(ctx, tc, aps):
    import concourse.bass as bass
    from concourse import mybir

    nc = tc.nc
    f32 = mybir.dt.float32
    bf16 = mybir.dt.bfloat16
    AX = mybir.AluOpType
    AF = mybir.ActivationFunctionType
    ts = bass.ts

    inp, rr, win, binc, cbp, cbn, woa, eye = (
        aps["inp"], aps["rr"], aps["win"], aps["binc"], aps["cbp"],
        aps["cbn"], aps["woa"], aps["eye"],
    )
    out = aps["out"]

    # ---- pools ----
    const = ctx.enter_context(tc.tile_pool(name="const", bufs=1))
    persist = ctx.enter_context(tc.tile_pool(name="persist", bufs=1))
    inpool = ctx.enter_context(tc.tile_pool(name="inpool", bufs=4))
    sqpool = ctx.enter_context(tc.tile_pool(name="sqpool", bufs=2))
    rlpool = ctx.enter_context(tc.tile_pool(name="rlpool", bufs=3))
    dgpool = ctx.enter_context(tc.tile_pool(name="dgpool", bufs=2))
    opool = ctx.enter_context(tc.tile_pool(name="opool", bufs=2))

    xpsum = ctx.enter_context(tc.tile_pool(name="xpsum", bufs=2, space="PSUM"))
    dpsum = ctx.enter_context(tc.tile_pool(name="dpsum", bufs=2, space="PSUM"))
    tpsum = ctx.enter_context(tc.tile_pool(name="tpsum", bufs=1, space="PSUM"))
    opsum = ctx.enter_context(tc.tile_pool(name="opsum", bufs=1, space="PSUM"))

    # ---- constants ----
    w0 = const.tile([128, D], bf16, tag="w0", name="w0")
    nc.sync.dma_start(w0[:], win[0:128, :])
    w1 = const.tile([128, D], bf16, tag="w1", name="w1")
    nc.sync.dma_start(w1[:], win[128:256, :])
    binc_sb = const.tile([D, 1], f32, tag="binc", name="binc_sb")
    nc.sync.dma_start(binc_sb[:], binc[:])
    cbp_sb = const.tile([D + 1, K], bf16, tag="cbp", name="cbp_sb")
    nc.sync.dma_start(cbp_sb[:], cbp[:])
    cbn_sb = const.tile([D + 1, K], bf16, tag="cbn", name="cbn_sb")
    nc.sync.dma_start(cbn_sb[:], cbn[:])
    woa_sb = const.tile([D + 1, DIM], bf16, tag="woa", name="woa_sb")
    nc.sync.dma_start(woa_sb[:], woa[:])
    eye_sb = const.tile([128, 128], bf16, tag="eye", name="eye_sb")
    nc.sync.dma_start(eye_sb[:], eye[:])

    # normalized random vectors, token-major [128, NCHUNK, 64]
    rall = persist.tile([128, NCHUNK, D], bf16, tag="rall", name="rall")
    nc.sync.dma_start(rall[:], rr[:])

    # persistent x-hat / q-hat tiles with a ones row at row 64
    xh = [persist.tile([D + 1, TTILE], bf16, tag=f"xh{n}", name=f"xh{n}")
          for n in range(4)]
    qh = [persist.tile([D + 1, TTILE], bf16, tag=f"qh{n}", name=f"qh{n}")
          for n in range(4)]
    for t_ in xh + qh:
        nc.gpsimd.memset(t_[D:D + 1, :], 1.0)

    stats = {}
    for nm in ("smax", "nsq", "resid2", "scalev"):
        stats[nm] = persist.tile([128, NCHUNK], f32, tag=nm, name=nm)
    smax, nsq = stats["smax"], stats["nsq"]
    resid2, scalev = stats["resid2"], stats["scalev"]

    Xtiles = {}

    def phase_a(i):
        b, t4 = divmod(i, NTILES // BPC)
        t0 = t4 * TTILE
        xt = xh[i % 4]

        in0 = inpool.tile([128, TTILE], bf16, tag="in0", name="in0")
        nc.sync.dma_start(in0[:], inp[b, 0:128, t0:t0 + TTILE])
        in1 = inpool.tile([128, TTILE], bf16, tag="in1", name="in1")
        nc.sync.dma_start(in1[:], inp[b, 128:256, t0:t0 + TTILE])

        X = xpsum.tile([D, TTILE], f32, tag="X", name="X")
        Xtiles[i] = X
        nc.tensor.matmul(X[:], w0[:], in0[:], start=True, stop=False)
        nc.tensor.matmul(X[:], w1[:], in1[:], start=False, stop=True)
        nc.scalar.activation(xt[0:D, :], X[:], AF.Identity, bias=binc_sb[:])

        # ||x||^2 from the bf16 xt (consistent with dist scores)
        if "nsq" not in ABLATE:
            XT4 = tpsum.tile([128, CPT, D], bf16, tag="xtm", name="XT4")
            for j4 in range(CPT):
                nc.tensor.transpose(XT4[:, j4, :], xt[0:D, ts(j4, CHUNK)],
                                    eye_sb[0:D, 0:D])
            sq4 = sqpool.tile([128, CPT, D], bf16, tag="sq4", name="sq4")
            nc.scalar.activation(sq4[:].rearrange("p c d -> p (c d)"),
                                 XT4[:].rearrange("p c d -> p (c d)"),
                                 AF.Square)
            nc.vector.reduce_sum(nsq[:, ts(i, CPT)], sq4[:],
                                 axis=mybir.AxisListType.X)

        for j4 in range(CPT):
            j = CPT * i + j4
            xsl = xt[0:D + 1, ts(j4, CHUNK)]
            dd = dpsum.tile([128, K], f32, tag="d", name="dd")
            if j4 in ACT_CHUNKS:
                # pair path: [s_even | s_diff], relu on ACT, add on PE
                nc.tensor.matmul(dd[:, 0:K // 2], xsl, cbp_sb[:, 0:K // 2],
                                 start=True, stop=True)
                nc.tensor.matmul(dd[:, K // 2:K], xsl, cbp_sb[:, K // 2:K],
                                 start=True, stop=True)
                rl = rlpool.tile([128, K // 2], bf16, tag="rl", name="rl")
                nc.scalar.activation(rl[:], dd[:, K // 2:K], AF.Relu)
                nc.tensor.matmul(dd[:, 0:K // 2], eye_sb[:], rl[:],
                                 start=False, stop=True,
                                 skip_group_check=True)
                nc.vector.reduce_max(smax[:, j:j + 1], dd[:, 0:K // 2],
                                     axis=mybir.AxisListType.X)
            else:
                # plain path: full codebook, one wide DVE reduce
                nc.tensor.matmul(dd[:, 0:K // 2], xsl, cbn_sb[:, 0:K // 2],
                                 start=True, stop=True)
                nc.tensor.matmul(dd[:, K // 2:K], xsl, cbn_sb[:, K // 2:K],
                                 start=True, stop=True)
                nc.vector.reduce_max(smax[:, j:j + 1], dd[:],
                                     axis=mybir.AxisListType.X)

    def scale_math(i):
        if "smath" in ABLATE:
            return
        c4 = ts(i, CPT)
        nc.vector.scalar_tensor_tensor(resid2[:, c4], smax[:, c4], -2.0,
                                       nsq[:, c4], AX.mult, AX.add)
        nc.vector.tensor_scalar_max(resid2[:, c4], resid2[:, c4], 0.0)
        nc.scalar.sqrt(scalev[:, c4], resid2[:, c4])

    def phase_b(i):
        b, t4 = divmod(i, NTILES // BPC)
        t0 = t4 * TTILE
        X = Xtiles.pop(i)
        qt = qh[i % 4]

        # q^T = x^T + rhat_chunk^T @ diag(scale), accumulated onto X;
        # diag(scale) built on gpsimd from the bf16 eye
        dg = dgpool.tile([128, CPT, 128], bf16, tag="dg", name="dg")
        for j4 in range(CPT):
            j = CPT * i + j4
            nc.gpsimd.tensor_scalar_mul(dg[:, j4, :], eye_sb[:],
                                        scalev[:, j:j + 1])
            nc.tensor.matmul(X[:, ts(j4, CHUNK)], rall[:, j, :], dg[:, j4, :],
                             start=False, stop=(j4 == CPT - 1),
                             skip_group_check=True)
        nc.scalar.activation(qt[0:D, :], X[:], AF.Identity, bias=binc_sb[:])

        osb = opool.tile([128, 2, TTILE], bf16, tag="osb", name="osb")
        for m in range(2):
            O = opsum.tile([128, TTILE], f32, tag="O", name="O")
            nc.tensor.matmul(O[:], woa_sb[:, ts(m, 128)], qt[:],
                             start=True, stop=True)
            if OSB_ENG[m] == "v":
                nc.vector.tensor_copy(osb[:, m, :], O[:])
            else:
                nc.scalar.activation(osb[:, m, :], O[:], AF.Identity)
        nc.sync.dma_start(out[b, 0:128, t0:t0 + TTILE], osb[:, 0, :])
        nc.sync.dma_start(out[b, 128:256, t0:t0 + TTILE], osb[:, 1, :])

    for _rep in range(int(os.environ.get("KREPEAT", "1"))):
        for i in range(NTILES):
            phase_a(i)
            scale_math(i)
            phase_b(i)


def build():
    if "nc" in _CACHE:
        return _CACHE["nc"]
    from concourse import bacc, mybir
    import concourse.tile as tile

    nc = bacc.Bacc("TRN2", target_bir_lowering=False, debug=False,
                   enable_asserts=False, num_devices=NCORES)
    f32 = mybir.dt.float32
    bf16 = mybir.dt.bfloat16
    aps = {
        "inp": nc.dram_tensor("inp", [BPC, DIM, T], bf16,
                              kind="ExternalInput").ap(),
        "rr": nc.dram_tensor("rr", [128, NCHUNK, D], bf16,
                             kind="ExternalInput").ap(),
        "win": nc.dram_tensor("win", [DIM, D], bf16,
                              kind="ExternalInput").ap(),
        "binc": nc.dram_tensor("binc", [D, 1], f32,
                               kind="ExternalInput").ap(),
        "cbp": nc.dram_tensor("cbp", [D + 1, K], bf16,
                              kind="ExternalInput").ap(),
        "cbn": nc.dram_tensor("cbn", [D + 1, K], bf16,
                              kind="ExternalInput").ap(),
        "woa": nc.dram_tensor("woa", [D + 1, DIM], bf16,
                              kind="ExternalInput").ap(),
        "eye": nc.dram_tensor("eye", [128, 128], bf16,
                              kind="ExternalInput").ap(),
        "out": nc.dram_tensor("out", [BPC, DIM, T], bf16,
                              kind="ExternalOutput").ap(),
    }
    with tile.TileContext(nc) as tc:
        with ExitStack() as ctx:
            _emit(ctx, tc, aps)
    nc.compile()
    _CACHE["nc"] = nc
    return nc


def make_in_maps(input_data, codebooks, W_in, b_in, W_out, b_out,
                 random_vector):
    import ml_dtypes
    f = np.float32
    bf = ml_dtypes.bfloat16
    cb = np.asarray(codebooks, f)
    h = -0.5 * (cb * cb).sum(1)  # [K]
    # plain augmented codebook [65, K]
    cbn = np.concatenate([cb.T, h[None, :]], 0).astype(bf)
    # paired codebook: [even | (odd - even)] with matching bias rows
    ce, co = cb[0::2], cb[1::2]           # [512, 64] each
    he, ho = h[0::2], h[1::2]
    cbp = np.concatenate([
        np.concatenate([ce.T, he[None, :]], 0),
        np.concatenate([(co - ce).T, (ho - he)[None, :]], 0)], 1).astype(bf)
    woa = np.concatenate([np.asarray(W_out, f),
                          np.asarray(b_out, f)[None, :]], 0).astype(bf)
    eye = np.eye(128, dtype=bf)
    binc = np.ascontiguousarray(np.asarray(b_in, f).reshape(D, 1))
    win = np.asarray(W_in, f).astype(bf)
    rv = np.asarray(random_vector, f)
    rhat = rv / (np.sqrt((rv * rv).sum(1, keepdims=True)) + EPS)
    rhat = rhat.astype(bf).reshape(NCORES, NCHUNK, 128, D)
    inp_bf = np.asarray(input_data, f).astype(bf)
    in_maps = []
    for i in range(NCORES):
        rr = np.ascontiguousarray(rhat[i].transpose(1, 0, 2))
        in_maps.append({
            "inp": np.ascontiguousarray(inp_bf[BPC * i:BPC * (i + 1)]),
            "rr": rr,
            "win": win, "binc": binc, "cbp": cbp, "cbn": cbn, "woa": woa,
            "eye": eye,
        })
    return in_maps


def kernel(input_data, codebooks, W_in, b_in, W_out, b_out, random_vector,
           **kwargs):
    from concourse.bass_utils import run_bass_kernel_spmd

    nc = build()
    in_maps = make_in_maps(input_data, codebooks, W_in, b_in, W_out, b_out,
                           random_vector)
    res = run_bass_kernel_spmd(nc, in_maps, core_ids=list(range(NCORES)),
                               **kwargs)
    out = np.concatenate(
        [np.asarray(res.results[i]["out"]).astype(np.float32)
         for i in range(NCORES)], axis=0)
    _CACHE["last_res"] = res
    return out


if __name__ == "__main__":
    nc = build()
    print("compiled OK")


# revision 49
# speedup vs baseline: 1.1403x; 1.1298x over previous
"""NSVQ (noise-substitution VQ) Trainium2 kernel, v4.

out = decode(x + ||x - c_nearest|| * rhat), rhat = r/(||r||+eps) host-
precomputed, x = encode(input). ||x - c_n||^2 = ||x||^2 - 2 smax where
smax = max_k (x.c_k - 0.5||c_k||^2) -- no argmin / gather needed.

Max strategy (DVE is the only engine that can max; gpsimd/Pool ALU only
implements Add/Multiply; TensorTensor with 2 PSUM operands is rejected
by walrus): use the pair identity
    max(s_even, s_odd) = s_even + relu(s_odd - s_even)
where s_odd - s_even is linear in [x; 1], so the dist matmul emits
[s_even | s_diff] via a pre-paired codebook. ACT computes relu(s_diff)
(PSUM -> SBUF bf16), PE accumulates it back onto the s_even PSUM half
(eye matmul, start=False), DVE does one 512-wide reduce_max. Chunks can
alternatively take the plain path (full 1024-wide DVE reduce_max of an
unpaired codebook) -- the KACT_CHUNKS set balances ACT vs DVE load.

Everything bf16 on-chip (inputs cast on host); output DMA'd bf16 and
cast back to f32 on host. Data-parallel over tokens: core i handles
batches [2i, 2i+1]; codebook + projection weights replicated.
"""

import numpy as np
from contextlib import ExitStack

B, DIM, T = 16, 256, 2048
K, D = 1024, 64
NCORES = 8
BPC = B // NCORES          # batches per core
NTOK = BPC * T             # tokens per core
TTILE = 512                # tokens per tile
NTILES = NTOK // TTILE     # 8
CHUNK = 128
CPT = TTILE // CHUNK       # chunks per tile = 4
NCHUNK = NTOK // CHUNK     # 32
EPS = 1e-12

_CACHE = {}

import os
ABLATE = set(os.environ.get("KABLATE", "").split(",")) - {""}
# chunk indices (within tile) using the ACT-relu pair path; rest use the
# plain full-reduce path on DVE
ACT_CHUNKS = set(int(c) for c in os.environ.get("KACT_CHUNKS", "0,2").split(",")
                 if c != "")
# osb evacuation engines for the two output halves: a=ACT, v=DVE
OSB_ENG = os.environ.get("KOSB", "aa")


def _emit(ctx, tc, aps):
    import concourse.bass as bass
    from concourse import mybir

    nc = tc.nc
    f32 = mybir.dt.float32
    bf16 = mybir.dt.bfloat16
    AX = mybir.AluOpType
    AF = mybir.ActivationFunctionType
    ts = bass.ts

    inp, rr, win, binc, cbp, cbn, woa, eye = (
        aps["inp"], aps["rr"], aps["win"], aps["binc"], aps["cbp"],
        aps["cbn"], aps["woa"], aps["eye"],
    )
    out = aps["out"]

    # ---- pools ----
    const = ctx.enter_context(tc.tile_pool(name="const", bufs=1))
    persist = ctx.enter_context(tc.tile_pool(name="persist", bufs=1))
    inpool = ctx.enter_context(tc.tile_pool(name="inpool", bufs=4))
    sqpool = ctx.enter_context(tc.tile_pool(name="sqpool", bufs=2))
    rlpool = ctx.enter_context(tc.tile_pool(name="rlpool", bufs=3))
    dgpool = ctx.enter_context(tc.tile_pool(name="dgpool", bufs=2))
    opool = ctx.enter_context(tc.tile_pool(name="opool", bufs=2))

    EB = int(os.environ.get("KEBUFS", "2"))
    DB = int(os.environ.get("KDBUFS", "2"))
    OB = int(os.environ.get("KOBUFS", "1"))
    xpsum = ctx.enter_context(tc.tile_pool(name="xpsum", bufs=2, space="PSUM"))
    epsum = ctx.enter_context(tc.tile_pool(name="epsum", bufs=EB, space="PSUM"))
    dpsum = ctx.enter_context(tc.tile_pool(name="dpsum", bufs=DB, space="PSUM"))
    tpsum = ctx.enter_context(tc.tile_pool(name="tpsum", bufs=1, space="PSUM"))
    opsum = ctx.enter_context(tc.tile_pool(name="opsum", bufs=OB, space="PSUM"))

    # ---- constants on the scalar (ACT) DGE queue so the sync (SP) queue
    # starts on tile 0's inputs immediately ----
    w0 = const.tile([128, D], bf16, tag="w0", name="w0")
    nc.scalar.dma_start(w0[:], win[0:128, :])
    w1 = const.tile([128, D], bf16, tag="w1", name="w1")
    nc.scalar.dma_start(w1[:], win[128:256, :])
    binc_sb = const.tile([D, 1], f32, tag="binc", name="binc_sb")
    nc.scalar.dma_start(binc_sb[:], binc[:])
    eye_sb = const.tile([128, 128], bf16, tag="eye", name="eye_sb")
    nc.scalar.dma_start(eye_sb[:], eye[:])
    cbp_sb = const.tile([D + 1, K], bf16, tag="cbp", name="cbp_sb")
    nc.scalar.dma_start(cbp_sb[:], cbp[:])
    woa_sb = const.tile([D + 1, DIM], bf16, tag="woa", name="woa_sb")
    nc.scalar.dma_start(woa_sb[:], woa[:])

    # normalized random vectors, token-major [128, NCHUNK, 64]; DMA'd
    # lazily per tile to keep the prologue queues short
    rall = persist.tile([128, NCHUNK, D], bf16, tag="rall", name="rall")

    # persistent x-hat / q-hat tiles with a ones row at row 64
    xh = [persist.tile([D + 1, TTILE], bf16, tag=f"xh{n}", name=f"xh{n}")
          for n in range(4)]
    qh = [persist.tile([D + 1, TTILE], bf16, tag=f"qh{n}", name=f"qh{n}")
          for n in range(4)]
    for t_ in xh + qh:
        nc.gpsimd.memset(t_[D:D + 1, :], 1.0)

    # per-tile stats in a rotating pool (avoids cross-tile false deps)
    statpool = ctx.enter_context(tc.tile_pool(name="statpool", bufs=3))
    Stiles = {}

    Xtiles = {}
    Btiles = {}
    Dtiles = {}

    def phase_a(i):
        b, t4 = divmod(i, NTILES // BPC)
        t0 = t4 * TTILE
        xt = xh[i % 4]

        nc.sync.dma_start(rall[:, ts(i, CPT), :], rr[:, ts(i, CPT), :])
        in01 = inpool.tile([128, 2, TTILE], bf16, tag="in01", name="in01")
        nc.sync.dma_start(
            in01[:],
            inp[b, :, t0:t0 + TTILE].rearrange("(a p) t -> p a t", p=128))

        X = xpsum.tile([D, TTILE], f32, tag="X", name="X")
        Xtiles[i] = X
        nc.tensor.matmul(X[:], w0[:], in01[:, 0, :], start=True, stop=False)
        nc.tensor.matmul(X[:], w1[:], in01[:, 1, :], start=False, stop=True)
        nc.scalar.activation(xt[0:D, :], X[:], AF.Identity, bias=binc_sb[:])

        sm = statpool.tile([128, 2, CPT], f32, tag="sm", name="sm")
        nsq4 = sm[:, 1, :]
        Stiles[i] = sm

        # ||x||^2 from the bf16 xt (consistent with dist scores)
        if "nsq" not in ABLATE:
            XT4 = tpsum.tile([128, CPT, D], bf16, tag="xtm", name="XT4")
            for j4 in range(CPT):
                nc.tensor.transpose(XT4[:, j4, :], xt[0:D, ts(j4, CHUNK)],
                                    eye_sb[0:D, 0:D])
            sq4 = sqpool.tile([128, CPT, D], bf16, tag="sq4", name="sq4")
            nc.scalar.activation(sq4[:].rearrange("p c d -> p (c d)"),
                                 XT4[:].rearrange("p c d -> p (c d)"),
                                 AF.Square)
            nc.vector.reduce_sum(nsq4[:], sq4[:], axis=mybir.AxisListType.X)

        # All chunks use the pair path: dd_e = s_even, dd_d = s_diff in
        # separate PSUM banks (dd_d recycles right after its relu). The
        # relu engine alternates ACT / DVE by chunk to balance load. The
        # PE pair-add and DVE reduce are deferred by one chunk so the PE
        # FIFO never waits on relu latency for the freshest chunk.
        pend = []

        def finish(j4, de, rl):
            if "add" not in ABLATE and "relu" not in ABLATE:
                nc.tensor.matmul(de[:], eye_sb[:], rl[:], start=False,
                                 stop=True, skip_group_check=True)
            if "reduce" not in ABLATE:
                nc.vector.reduce_max(sm[:, 0, j4:j4 + 1], de[:],
                                     axis=mybir.AxisListType.X)

        for j4 in range(CPT):
            xsl = xt[0:D + 1, ts(j4, CHUNK)]
            de = epsum.tile([128, K // 2], f32, tag="e", name="de")
            dd = dpsum.tile([128, K // 2], f32, tag="d", name="dd")
            nc.tensor.matmul(de[:], xsl, cbp_sb[:, 0:K // 2],
                             start=True, stop=True)
            nc.tensor.matmul(dd[:], xsl, cbp_sb[:, K // 2:K],
                             start=True, stop=True)
            rl = rlpool.tile([128, K // 2], bf16, tag="rl", name="rl")
            if "relu" not in ABLATE:
                if j4 in ACT_CHUNKS:
                    nc.scalar.activation(rl[:], dd[:], AF.Relu)
                else:
                    nc.vector.tensor_scalar_max(rl[:], dd[:], 0.0)
            pend.append((j4, de, rl))
            if len(pend) > 1:
                finish(*pend.pop(0))
        while pend:
            finish(*pend.pop(0))

    def scale_math(i):
        if "smath" in ABLATE:
            return
        sm = Stiles.pop(i)
        sv = statpool.tile([128, 2, CPT], f32, tag="sv", name="sv")
        resid2, scalev = sv[:, 0, :], sv[:, 1, :]
        nc.vector.scalar_tensor_tensor(resid2, sm[:, 0, :], -2.0,
                                       sm[:, 1, :], AX.mult, AX.add)
        nc.vector.tensor_scalar_max(resid2, resid2, 0.0)
        nc.scalar.sqrt(scalev, resid2)
        # pre-build the vq scale diagonals so phase_b's taccum matmuls
        # find them ready one pipeline step later
        dg = dgpool.tile([128, CPT, 128], bf16, tag="dg", name="dg")
        Dtiles[i] = dg
        for j4 in range(CPT):
            nc.gpsimd.tensor_scalar_mul(dg[:, j4, :], eye_sb[:],
                                        scalev[:, j4:j4 + 1])

    def phase_b(i):
        b, t4 = divmod(i, NTILES // BPC)
        t0 = t4 * TTILE
        X = Xtiles.pop(i)
        qt = qh[i % 4]

        # q^T = x^T + rhat_chunk^T @ diag(scale), accumulated onto X
        dg = Dtiles.pop(i)
        for j4 in range(CPT):
            j = CPT * i + j4
            nc.tensor.matmul(X[:, ts(j4, CHUNK)], rall[:, j, :], dg[:, j4, :],
                             start=False, stop=(j4 == CPT - 1),
                             skip_group_check=True)
        nc.scalar.activation(qt[0:D, :], X[:], AF.Identity, bias=binc_sb[:])

        osb = opool.tile([128, 2, TTILE], bf16, tag="osb", name="osb")
        O = opsum.tile([128, TTILE], f32, tag="O", name="O")
        nc.tensor.matmul(O[:], woa_sb[:, ts(0, 128)], qt[:],
                         start=True, stop=True)
        if OSB_ENG[0] == "v":
            nc.vector.tensor_copy(osb[:, 0, :], O[:])
        else:
            nc.scalar.activation(osb[:, 0, :], O[:], AF.Identity)
        Btiles[i] = (b, t0, qt, osb)

    def phase_b2(i):
        b, t0, qt, osb = Btiles.pop(i)
        O = opsum.tile([128, TTILE], f32, tag="O", name="O")
        nc.tensor.matmul(O[:], woa_sb[:, ts(1, 128)], qt[:],
                         start=True, stop=True)
        if OSB_ENG[1] == "v":
            nc.vector.tensor_copy(osb[:, 1, :], O[:])
        else:
            nc.scalar.activation(osb[:, 1, :], O[:], AF.Identity)
        outq = {"sync": nc.sync, "scalar": nc.scalar,
                "gpsimd": nc.gpsimd}[os.environ.get("KOUTQ", "sync")]
        outq.dma_start(
            out[b, :, t0:t0 + TTILE].rearrange("(a p) t -> p a t", p=128),
            osb[:])

    # software pipeline: phase_b of tile i-1 (and its second output half,
    # phase_b2, of tile i-2) are emitted after phase_a of tile i, so bulk
    # ACT work queues behind tile i's latency-critical relus, and the
    # decode mm1 -> evac1 chain never head-of-line-blocks either FIFO
    for _rep in range(int(os.environ.get("KREPEAT", "1"))):
        for i in range(NTILES):
            phase_a(i)
            scale_math(i)
            if i >= 2:
                phase_b2(i - 2)
            if i >= 1:
                phase_b(i - 1)
        phase_b2(NTILES - 2)
        phase_b(NTILES - 1)
        phase_b2(NTILES - 1)


def build():
    if "nc" in _CACHE:
        return _CACHE["nc"]
    from concourse import bacc, mybir
    import concourse.tile as tile

    nc = bacc.Bacc("TRN2", target_bir_lowering=False, debug=False,
                   enable_asserts=False, num_devices=NCORES)
    f32 = mybir.dt.float32
    bf16 = mybir.dt.bfloat16
    aps = {
        "inp": nc.dram_tensor("inp", [BPC, DIM, T], bf16,
                              kind="ExternalInput").ap(),
        "rr": nc.dram_tensor("rr", [128, NCHUNK, D], bf16,
                             kind="ExternalInput").ap(),
        "win": nc.dram_tensor("win", [DIM, D], bf16,
                              kind="ExternalInput").ap(),
        "binc": nc.dram_tensor("binc", [D, 1], f32,
                               kind="ExternalInput").ap(),
        "cbp": nc.dram_tensor("cbp", [D + 1, K], bf16,
                              kind="ExternalInput").ap(),
        "cbn": nc.dram_tensor("cbn", [D + 1, K], bf16,
                              kind="ExternalInput").ap(),
        "woa": nc.dram_tensor("woa", [D + 1, DIM], bf16,
                              kind="ExternalInput").ap(),
        "eye": nc.dram_tensor("eye", [128, 128], bf16,
                              kind="ExternalInput").ap(),
        "out": nc.dram_tensor("out", [BPC, DIM, T], bf16,
                              kind="ExternalOutput").ap(),
    }
    with tile.TileContext(nc) as tc:
        with ExitStack() as ctx:
            _emit(ctx, tc, aps)
    nc.compile()
    _CACHE["nc"] = nc
    return nc


def make_in_maps(input_data, codebooks, W_in, b_in, W_out, b_out,
                 random_vector):
    import ml_dtypes
    f = np.float32
    bf = ml_dtypes.bfloat16
    cb = np.asarray(codebooks, f)
    h = -0.5 * (cb * cb).sum(1)  # [K]
    # plain augmented codebook [65, K]
    cbn = np.concatenate([cb.T, h[None, :]], 0).astype(bf)
    # paired codebook: [even | (odd - even)] with matching bias rows
    ce, co = cb[0::2], cb[1::2]           # [512, 64] each
    he, ho = h[0::2], h[1::2]
    cbp = np.concatenate([
        np.concatenate([ce.T, he[None, :]], 0),
        np.concatenate([(co - ce).T, (ho - he)[None, :]], 0)], 1).astype(bf)
    woa = np.concatenate([np.asarray(W_out, f),
                          np.asarray(b_out, f)[None, :]], 0).astype(bf)
    eye = np.eye(128, dtype=bf)
    binc = np.ascontiguousarray(np.asarray(b_in, f).reshape(D, 1))
    win = np.asarray(W_in, f).astype(bf)
    rv = np.asarray(random_vector, f)
    rhat = rv / (np.sqrt((rv * rv).sum(1, keepdims=True)) + EPS)
    rhat = rhat.astype(bf).reshape(NCORES, NCHUNK, 128, D)
    inp_bf = np.asarray(input_data, f).astype(bf)
    in_maps = []
    for i in range(NCORES):
        rr = np.ascontiguousarray(rhat[i].transpose(1, 0, 2))
        in_maps.append({
            "inp": np.ascontiguousarray(inp_bf[BPC * i:BPC * (i + 1)]),
            "rr": rr,
            "win": win, "binc": binc, "cbp": cbp, "cbn": cbn, "woa": woa,
            "eye": eye,
        })
    return in_maps


def kernel(input_data, codebooks, W_in, b_in, W_out, b_out, random_vector,
           **kwargs):
    from concourse.bass_utils import run_bass_kernel_spmd

    nc = build()
    in_maps = make_in_maps(input_data, codebooks, W_in, b_in, W_out, b_out,
                           random_vector)
    res = run_bass_kernel_spmd(nc, in_maps, core_ids=list(range(NCORES)),
                               **kwargs)
    out = np.concatenate(
        [np.asarray(res.results[i]["out"]).astype(np.float32)
         for i in range(NCORES)], axis=0)
    _CACHE["last_res"] = res
    return out


if __name__ == "__main__":
    nc = build()
    print("compiled OK")
